# revision 25
# baseline (speedup 1.0000x reference)
"""CapsuleLayer dynamic-routing kernel for 8 Trainium2 NeuronCores.

Math (per routing iteration, reassociated to avoid materializing low_new):
    W~[b,k,l]   = exp(Bc[k,l] + mask[b,l] - max_l)          (unnormalized)
    mid[b,k,i]  = sum_l W~[b,k,l] low_cap[b,l,i] / sum_l W~  (1/sum folded here)
    high[b,k,o] = squash_o( sum_i mid[b,k,i] S[i,o] )
    delta[k,l]  = sum_{b,i} (high@S^T)[b,k,i] low_cap[b,l,i]   -> AllReduce(8 cores)
    Bc         += delta        (iterations 1,2 only; iteration 3 emits high)

Sharding: data-parallel over batch (512/core). Host pre-stages low_capsule in
two layouts (l-major for the l-contraction, i-major for the i-contraction)
because f32 DMA transpose does not exist on trn2.
"""
import os
import sys

for _p in ("/opt/trn_rl_repo", "/root/.axon_site/_ro/trn_rl_repo"):
    if os.path.isdir(_p) and _p not in sys.path:
        sys.path.insert(0, _p)

import numpy as np

import concourse.bacc as bacc
import concourse.tile as tile
from concourse import mybir
from concourse import bass_utils
from concourse.masks import make_identity

# problem shape (hardcoded per contract)
B, L, DIN, DOUT, K = 4096, 200, 64, 64, 8
NCORES = 8
BS = B // NCORES           # 512 batches per core
LP = 224                   # L padded to /32 (128 + 96 partition chunks)
L0, L1 = 128, 96
TB = 16                    # batches per half-tile -> TB*K = 128 partitions
SUPER = 32                 # batches per DMA super-tile
NSUPER = BS // SUPER       # 16
PADV = -2.0 ** 16 + 1.0    # the reference's masked-logit value (finite!)
ITERS = 3

F32 = mybir.dt.float32


def _build_program(bs=BS, iters=ITERS, ncores=NCORES, debug_dumps=False):
    nc = bacc.Bacc(
        "TRN2",
        target_bir_lowering=False,
        debug=False,
        enable_asserts=False,
        num_devices=ncores,
    )
    lcL = nc.dram_tensor("lcL", [LP, bs, DIN], F32, kind="ExternalInput").ap()
    lcT = nc.dram_tensor("lcT", [DIN, bs, LP], F32, kind="ExternalInput").ap()
    maskBK = nc.dram_tensor("maskBK", [bs * K, LP], mybir.dt.uint8, kind="ExternalInput").ap()
    Bc0 = nc.dram_tensor("Bc0", [TB * K, LP], F32, kind="ExternalInput").ap()
    S_in = nc.dram_tensor("S", [DIN, DOUT], F32, kind="ExternalInput").ap()
    ST_in = nc.dram_tensor("ST", [DOUT, DIN], F32, kind="ExternalInput").ap()
    out = nc.dram_tensor("high", [bs, K, DOUT], F32, kind="ExternalOutput").ap()
    dumps = None
    if debug_dumps:
        dumps = [
            nc.dram_tensor(f"d{i}_dump", [128, LP], F32, kind="ExternalOutput").ap()
            for i in range(iters - 1)
        ]

    from contextlib import ExitStack

    with tile.TileContext(nc) as tc, ExitStack() as ctx:
        _kernel_body(ctx, tc, out, lcL, lcT, maskBK, Bc0, S_in, ST_in,
                     bs=bs, iters=iters, ncores=ncores, dumps=dumps)
    nc.compile()
    return nc


def _kernel_body(ctx, tc, out, lcL, lcT, maskBK, Bc0, S_in, ST_in,
                 bs=BS, iters=ITERS, ncores=NCORES, dumps=None):
    nc = tc.nc
    ex = mybir.ActivationFunctionType
    NHALF = bs // TB  # half-tiles per iteration
    NSUP = bs // SUPER

    # ---------------- pools ----------------
    const = ctx.enter_context(tc.tile_pool(name="const", bufs=1))
    bcrep_pool = ctx.enter_context(tc.tile_pool(name="bcrep", bufs=2))
    lcA_pool = ctx.enter_context(tc.tile_pool(name="lcA", bufs=3))
    lcB_pool = ctx.enter_context(tc.tile_pool(name="lcB", bufs=3))
    lcT_pool = ctx.enter_context(tc.tile_pool(name="lcTp", bufs=2))
    work = ctx.enter_context(tc.tile_pool(name="work", bufs=3))
    small = ctx.enter_context(tc.tile_pool(name="small", bufs=4))
    dram = ctx.enter_context(tc.tile_pool(name="dram", bufs=2, space="DRAM"))
    # PSUM: 8 banks -> tr(2) + mid_big(2) + high(2) + gt_all(1) + delta(1)
    tr_ps = ctx.enter_context(tc.tile_pool(name="tr_ps", bufs=2, space="PSUM"))
    mb_ps = ctx.enter_context(tc.tile_pool(name="mb_ps", bufs=2, space="PSUM"))
    hi_ps = ctx.enter_context(tc.tile_pool(name="hi_ps", bufs=2, space="PSUM"))
    ga_ps = ctx.enter_context(tc.tile_pool(name="ga_ps", bufs=1, space="PSUM"))
    dl_ps = ctx.enter_context(tc.tile_pool(name="dl_ps", bufs=1, space="PSUM"))

    # ---------------- constants / residents ----------------
    ident = const.tile([128, 128], F32)
    make_identity(nc, ident)
    S_sb = const.tile([DIN, DOUT], F32)
    nc.sync.dma_start(out=S_sb, in_=S_in)
    ST_sb = const.tile([DOUT, DIN], F32)
    nc.sync.dma_start(out=ST_sb, in_=ST_in)
    # resident additive mask, [(b k)] rows tiled as [128, NHALF, LP]
    mask_res = const.tile([128, NHALF, LP], mybir.dt.uint8)
    nc.sync.dma_start(
        out=mask_res, in_=maskBK.rearrange("(t p) l -> p t l", p=128)
    )
    eps_t = const.tile([128, 1], F32)
    nc.vector.memset(eps_t, 1e-9)
    # exact reference masking: masked real columns get PAD (finite! the
    # reference's where(mask, Bc, -65535) lets PAD positions win the softmax
    # when Bc drops below -65535, and we must reproduce that); the l>=200
    # padding columns get a huge negative so they never receive weight.
    pad_t = const.tile([128, LP], F32)
    nc.vector.memset(pad_t[:, 0:L], PADV)
    nc.vector.memset(pad_t[:, L:LP], -3.0e38)

    bc_rep = bcrep_pool.tile([128, LP], F32)
    nc.sync.dma_start(out=bc_rep, in_=Bc0)

    for it in range(iters):
        last = it == iters - 1
        if not last:
            delta_ps = dl_ps.tile([K, LP], F32)
        first_acc = True

        for st in range(NSUP):
            b0 = st * SUPER
            tA = lcA_pool.tile([L0, SUPER, DIN], F32)
            nc.sync.dma_start(out=tA, in_=lcL[0:L0, b0 : b0 + SUPER, :])
            tB = lcB_pool.tile([L1, SUPER, DIN], F32)
            nc.sync.dma_start(out=tB, in_=lcL[L0:LP, b0 : b0 + SUPER, :])
            if not last:
                tT = lcT_pool.tile([DIN, SUPER, LP], F32)
                nc.sync.dma_start(out=tT, in_=lcT[:, b0 : b0 + SUPER, :])

            for h in range(2):
                ht = st * 2 + h  # half-tile index in [0, NHALF)
                # ---- masked softmax: unnormalized exp + row sums ----
                # logits = where(mask, Bc, PAD) exactly as the reference
                logits = work.tile([128, LP], F32, tag="logits")
                nc.vector.tensor_copy(out=logits, in_=pad_t)
                nc.vector.copy_predicated(
                    out=logits, mask=mask_res[:, ht, :], data=bc_rep
                )
                negmax = small.tile([128, 1], F32, tag="negmax")
                nc.vector.reduce_max(
                    out=negmax, in_=logits, axis=mybir.AxisListType.X, negate=True
                )
                wexp = work.tile([128, LP], F32, tag="wexp")
                sumw = small.tile([128, 1], F32, tag="sumw")
                nc.scalar.activation(
                    out=wexp, in_=logits, func=ex.Exp, bias=negmax, scale=1.0,
                    accum_out=sumw,
                )
                rcs = small.tile([128, 1], F32, tag="rcs")
                nc.vector.reciprocal(out=rcs, in_=sumw)

                # ---- W~^T via PE transposes ----
                wt_a_ps = tr_ps.tile([128, 128], F32, tag="tr")
                nc.tensor.transpose(wt_a_ps, wexp[:, 0:L0], ident)
                wt_a = work.tile([128, 128], F32, tag="wt_a")
                nc.vector.tensor_copy(out=wt_a, in_=wt_a_ps)
                wt_b_ps = tr_ps.tile([128, 128], F32, tag="tr")
                nc.tensor.transpose(wt_b_ps[0:L1, :], wexp[:, L0:LP], ident)
                wt_b = work.tile([L1, 128], F32, tag="wt_b")
                nc.vector.tensor_copy(out=wt_b, in_=wt_b_ps[0:L1, :])

                # ---- midT[i, (b k)] = sum_l lc[l,i] W~[k,l], per batch ----
                midT_ps = mb_ps.tile([DIN, 128], F32, tag="midT")
                for j in range(TB):
                    bj = h * TB + j
                    cols = slice(j * K, (j + 1) * K)
                    nc.tensor.matmul(
                        midT_ps[:, cols],
                        lhsT=tA[:, bj, :],
                        rhs=wt_a[:, cols],
                        start=(j == 0), stop=False,
                    )
                    nc.tensor.matmul(
                        midT_ps[:, cols],
                        lhsT=tB[:, bj, :],
                        rhs=wt_b[:, cols],
                        start=False, stop=(j == TB - 1),
                    )
                midT_sb = work.tile([DIN, 128], F32, tag="midT_sb")
                nc.vector.tensor_copy(out=midT_sb, in_=midT_ps)

                # ---- high_raw (unnormalized) = midT^T @ S ----
                high_ps = hi_ps.tile([128, DOUT], F32, tag="high")
                nc.tensor.matmul(
                    high_ps, lhsT=midT_sb, rhs=S_sb, start=True, stop=True
                )

                # ---- squash with softmax 1/sum folded in ----
                sq_scr = work.tile([128, DOUT], F32, tag="sq_scr")
                nsqU = small.tile([128, 1], F32, tag="nsqU")
                nc.scalar.activation(
                    out=sq_scr, in_=high_ps, func=ex.Square, accum_out=nsqU
                )
                rcs2 = small.tile([128, 1], F32, tag="rcs2")
                nc.vector.tensor_mul(rcs2, rcs, rcs)
                nsq = small.tile([128, 1], F32, tag="nsq")
                nc.vector.tensor_mul(nsq, nsqU, rcs2)
                t1 = small.tile([128, 1], F32, tag="t1")
                nc.scalar.activation(out=t1, in_=nsq, func=ex.Sqrt, bias=eps_t)
                t2 = small.tile([128, 1], F32, tag="t2")
                nc.vector.tensor_scalar_add(t2, nsq, 1.0)
                t3 = small.tile([128, 1], F32, tag="t3")
                nc.vector.tensor_mul(t3, t1, t2)
                rc2 = small.tile([128, 1], F32, tag="rc2")
                nc.vector.reciprocal(out=rc2, in_=t3)
                scl0 = small.tile([128, 1], F32, tag="scl0")
                nc.vector.tensor_mul(scl0, nsq, rc2)
                scl = small.tile([128, 1], F32, tag="scl")
                nc.vector.tensor_mul(scl, scl0, rcs)
                high_sb = work.tile([128, DOUT], F32, tag="high_sb")
                nc.vector.tensor_scalar_mul(high_sb, high_ps, scl)

                if last:
                    nc.sync.dma_start(
                        out=out[ht * TB : (ht + 1) * TB, :, :].rearrange(
                            "b k o -> (b k) o"
                        ),
                        in_=high_sb,
                    )
                else:
                    # ---- delta += (high @ S^T) @ lcT ----
                    highT_ps = tr_ps.tile([128, 128], F32, tag="tr")
                    nc.tensor.transpose(highT_ps[0:DOUT, :], high_sb, ident)
                    highT_sb = work.tile([DOUT, 128], F32, tag="highT_sb")
                    nc.vector.tensor_copy(out=highT_sb, in_=highT_ps[0:DOUT, :])
                    # GT_all[i, (b k)] = sum_o S[i,o] highT[o, (b k)] in one mm
                    gt_all_ps = ga_ps.tile([DIN, 128], F32, tag="gt_all")
                    nc.tensor.matmul(
                        gt_all_ps, lhsT=ST_sb, rhs=highT_sb, start=True, stop=True
                    )
                    gt_all = work.tile([DIN, 128], F32, tag="gt_all_sb")
                    nc.vector.tensor_copy(out=gt_all, in_=gt_all_ps)
                    for j in range(TB):
                        bj = h * TB + j
                        nc.tensor.matmul(
                            delta_ps,
                            lhsT=gt_all[:, j * K : (j + 1) * K],
                            rhs=tT[:, bj, :],
                            start=first_acc,
                            stop=(st == NSUP - 1 and h == 1 and j == TB - 1),
                        )
                        first_acc = False

        if not last:
            # ---- AllReduce delta, update replicated Bc ----
            import concourse.bass as bass

            delta_sb = work.tile([K, LP], F32, tag="delta_sb")
            nc.vector.tensor_copy(out=delta_sb, in_=delta_ps)
            ib_small = dram.tile([K, LP], F32, tag="ib_small")
            nc.sync.dma_start(out=ib_small, in_=delta_sb)
            # replicate 16x in DRAM so the AR result is already partition-replicated
            ib_rep = dram.tile([128, LP], F32, tag="ib_rep")
            src_rep = bass.AP(
                tensor=ib_small.tensor,
                offset=ib_small.offset,
                ap=[[0, TB]] + [list(d) for d in ib_small.ap],
            )
            nc.sync.dma_start(
                out=ib_rep.rearrange("(j k) l -> j k l", k=K), in_=src_rep
            )
            ob_rep = dram.tile([128, LP], F32, tag="ob_rep")
            nc.gpsimd.collective_compute(
                "AllReduce",
                mybir.AluOpType.add,
                replica_groups=[list(range(ncores))],
                ins=[ib_rep.opt()],
                outs=[ob_rep.opt()],
            )
            delta_rep = bcrep_pool.tile([128, LP], F32, tag="delta_rep")
            nc.sync.dma_start(out=delta_rep, in_=ob_rep)
            if dumps is not None:
                nc.sync.dma_start(out=dumps[it], in_=delta_rep)
            bc_next = bcrep_pool.tile([128, LP], F32)
            nc.vector.tensor_add(out=bc_next, in0=bc_rep, in1=delta_rep)
            bc_rep = bc_next


_NC_CACHE = None


def _get_nc():
    global _NC_CACHE
    if _NC_CACHE is None:
        _NC_CACHE = _build_program()
    return _NC_CACHE


def _stage_inputs(low_capsule, seq_len, B_matrix, S_matrix):
    lc = np.asarray(low_capsule, dtype=np.float32)
    sl = np.asarray(seq_len, dtype=np.int32).reshape(B)
    Bm = np.asarray(B_matrix, dtype=np.float32).reshape(K, L)
    Sm = np.ascontiguousarray(np.asarray(S_matrix, dtype=np.float32))

    Bc0 = np.zeros((K, LP), dtype=np.float32)
    Bc0[:, :L] = Bm
    Bc0 = np.tile(Bc0, (TB, 1))  # [128, LP] partition-replicated
    ST = np.ascontiguousarray(Sm.T)
    iota = np.arange(LP)

    in_maps = []
    for c in range(NCORES):
        lcs = lc[c * BS : (c + 1) * BS]            # [BS, L, DIN]
        sls = sl[c * BS : (c + 1) * BS]
        lcP = np.zeros((BS, LP, DIN), dtype=np.float32)
        lcP[:, :L, :] = lcs
        lcLh = np.ascontiguousarray(lcP.transpose(1, 0, 2))   # [LP, BS, DIN]
        lcTh = np.ascontiguousarray(lcP.transpose(2, 0, 1))   # [DIN, BS, LP]
        m01 = (iota[None, :] < sls[:, None]).astype(np.uint8)  # 1=valid
        maskBK = np.repeat(m01, K, axis=0)                       # [BS*K, LP]
        in_maps.append(
            dict(lcL=lcLh, lcT=lcTh, maskBK=maskBK, Bc0=Bc0, S=Sm, ST=ST)
        )
    return in_maps


def kernel(low_capsule, seq_len, B_matrix, S_matrix, _trace=False, _tmpdir=None):
    nc = _get_nc()
    in_maps = _stage_inputs(low_capsule, seq_len, B_matrix, S_matrix)
    res = bass_utils.run_bass_kernel_spmd(
        nc, in_maps, core_ids=list(range(NCORES)), trace=_trace, tmpdir=_tmpdir
    )
    outp = np.concatenate(
        [res.results[c]["high"] for c in range(NCORES)], axis=0
    ).astype(np.float32)
    if _trace:
        kernel.last_results = res
    return outp


# revision 28
# speedup vs baseline: 1.6674x; 1.6674x over previous
"""CapsuleLayer dynamic-routing kernel for 8 Trainium2 NeuronCores.

Math (per routing iteration, reassociated to avoid materializing low_new):
    W~[b,k,l]   = exp(Bc[k,l] + mask[b,l] - max_l)          (unnormalized)
    mid[b,k,i]  = sum_l W~[b,k,l] low_cap[b,l,i] / sum_l W~  (1/sum folded here)
    high[b,k,o] = squash_o( sum_i mid[b,k,i] S[i,o] )
    delta[k,l]  = sum_{b,i} (high@S^T)[b,k,i] low_cap[b,l,i]   -> AllReduce(8 cores)
    Bc         += delta        (iterations 1,2 only; iteration 3 emits high)

Sharding: data-parallel over batch (512/core). Host pre-stages low_capsule in
two layouts (l-major for the l-contraction, i-major for the i-contraction)
because f32 DMA transpose does not exist on trn2.
"""
import os
import sys

for _p in ("/opt/trn_rl_repo", "/root/.axon_site/_ro/trn_rl_repo"):
    if os.path.isdir(_p) and _p not in sys.path:
        sys.path.insert(0, _p)

import numpy as np

import concourse.bacc as bacc
import concourse.tile as tile
from concourse import mybir
from concourse import bass_utils
from concourse.masks import make_identity

# problem shape (hardcoded per contract)
B, L, DIN, DOUT, K = 4096, 200, 64, 64, 8
NCORES = 8
BS = B // NCORES           # 512 batches per core
LP = 224                   # L padded to /32 (128 + 96 partition chunks)
L0, L1 = 128, 96
TB = 16                    # batches per half-tile -> TB*K = 128 partitions
SUPER = 32                 # batches per DMA super-tile
NSUPER = BS // SUPER       # 16
PADV = -2.0 ** 16 + 1.0    # the reference's masked-logit value (finite!)
ITERS = 3

F32 = mybir.dt.float32


def _build_program(bs=BS, iters=ITERS, ncores=NCORES, debug_dumps=False):
    nc = bacc.Bacc(
        "TRN2",
        target_bir_lowering=False,
        debug=False,
        enable_asserts=False,
        num_devices=ncores,
    )
    lcL = nc.dram_tensor("lcL", [LP, bs, DIN], F32, kind="ExternalInput").ap()
    lcT = nc.dram_tensor("lcT", [2 * DIN, bs // 2, LP], F32, kind="ExternalInput").ap()
    maskBK = nc.dram_tensor("maskBK", [bs * K, LP], mybir.dt.uint8, kind="ExternalInput").ap()
    Bc0 = nc.dram_tensor("Bc0", [TB * K, LP], F32, kind="ExternalInput").ap()
    S_in = nc.dram_tensor("S", [DIN, DOUT], F32, kind="ExternalInput").ap()
    ST_in = nc.dram_tensor("ST", [DOUT, DIN], F32, kind="ExternalInput").ap()
    out = nc.dram_tensor("high", [bs, K, DOUT], F32, kind="ExternalOutput").ap()
    dumps = None
    if debug_dumps:
        dumps = [
            nc.dram_tensor(f"d{i}_dump", [128, LP], F32, kind="ExternalOutput").ap()
            for i in range(iters - 1)
        ]

    from contextlib import ExitStack

    with tile.TileContext(nc) as tc, ExitStack() as ctx:
        _kernel_body(ctx, tc, out, lcL, lcT, maskBK, Bc0, S_in, ST_in,
                     bs=bs, iters=iters, ncores=ncores, dumps=dumps)
    nc.compile()
    return nc


def _kernel_body(ctx, tc, out, lcL, lcT, maskBK, Bc0, S_in, ST_in,
                 bs=BS, iters=ITERS, ncores=NCORES, dumps=None):
    nc = tc.nc
    ex = mybir.ActivationFunctionType
    NHALF = bs // TB  # half-tiles per iteration
    NSUP = bs // SUPER

    # ---------------- pools ----------------
    const = ctx.enter_context(tc.tile_pool(name="const", bufs=1))
    bcrep_pool = ctx.enter_context(tc.tile_pool(name="bcrep", bufs=2))
    lcA_pool = ctx.enter_context(tc.tile_pool(name="lcA", bufs=3))
    lcB_pool = ctx.enter_context(tc.tile_pool(name="lcB", bufs=3))
    lcT_pool = ctx.enter_context(tc.tile_pool(name="lcTp", bufs=2))
    work = ctx.enter_context(tc.tile_pool(name="work", bufs=3))
    small = ctx.enter_context(tc.tile_pool(name="small", bufs=4))
    dram = ctx.enter_context(tc.tile_pool(name="dram", bufs=2, space="DRAM"))
    # PSUM: 8 banks -> tr(2) + midT(2) + high(1) + gt_dup(1) + delta(2)
    tr_ps = ctx.enter_context(tc.tile_pool(name="tr_ps", bufs=2, space="PSUM"))
    mb_ps = ctx.enter_context(tc.tile_pool(name="mb_ps", bufs=2, space="PSUM"))
    hi_ps = ctx.enter_context(tc.tile_pool(name="hi_ps", bufs=1, space="PSUM"))
    ga_ps = ctx.enter_context(tc.tile_pool(name="ga_ps", bufs=1, space="PSUM"))
    dl_ps = ctx.enter_context(tc.tile_pool(name="dl_ps", bufs=1, space="PSUM"))

    # ---------------- constants / residents ----------------
    ident = const.tile([128, 128], F32)
    make_identity(nc, ident)
    S_sb = const.tile([DIN, DOUT], F32)
    nc.sync.dma_start(out=S_sb, in_=S_in)
    ST_sb = const.tile([DOUT, DIN], F32)
    nc.sync.dma_start(out=ST_sb, in_=ST_in)
    # resident additive mask, [(b k)] rows tiled as [128, NHALF, LP]
    mask_res = const.tile([128, NHALF, LP], mybir.dt.uint8)
    nc.sync.dma_start(
        out=mask_res, in_=maskBK.rearrange("(t p) l -> p t l", p=128)
    )
    eps_t = const.tile([128, 1], F32)
    nc.vector.memset(eps_t, 1e-9)
    # exact reference masking: masked real columns get PAD (finite! the
    # reference's where(mask, Bc, -65535) lets PAD positions win the softmax
    # when Bc drops below -65535, and we must reproduce that); the l>=200
    # padding columns get a huge negative so they never receive weight.
    pad_t = const.tile([128, LP], F32)
    nc.vector.memset(pad_t[:, 0:L], PADV)
    nc.vector.memset(pad_t[:, L:LP], -3.0e38)

    bc_rep = bcrep_pool.tile([128, LP], F32)
    nc.sync.dma_start(out=bc_rep, in_=Bc0)

    for it in range(iters):
        last = it == iters - 1
        if not last:
            delta_psA = dl_ps.tile([K, LP], F32, tag="dlA")
            delta_psB = dl_ps.tile([K, LP], F32, tag="dlB")
        first_accA = True
        first_accB = True
        npairs_tot = (bs // 2)

        for st in range(NSUP):
            b0 = st * SUPER
            tA = lcA_pool.tile([L0, SUPER, DIN], F32)
            nc.sync.dma_start(out=tA, in_=lcL[0:L0, b0 : b0 + SUPER, :])
            tB = lcB_pool.tile([L1, SUPER, DIN], F32)
            nc.sync.dma_start(out=tB, in_=lcL[L0:LP, b0 : b0 + SUPER, :])
            if not last:
                tT = lcT_pool.tile([2 * DIN, SUPER // 2, LP], F32)
                nc.sync.dma_start(
                    out=tT, in_=lcT[:, st * (SUPER // 2) : (st + 1) * (SUPER // 2), :]
                )

            for h in range(2):
                ht = st * 2 + h  # half-tile index in [0, NHALF)
                # ---- masked softmax: unnormalized exp + row sums ----
                # logits = where(mask, Bc, PAD) exactly as the reference
                logits = work.tile([128, LP], F32, tag="logits")
                nc.vector.tensor_copy(out=logits, in_=pad_t)
                nc.vector.copy_predicated(
                    out=logits, mask=mask_res[:, ht, :], data=bc_rep
                )
                negmax = small.tile([128, 1], F32, tag="negmax")
                nc.vector.reduce_max(
                    out=negmax, in_=logits, axis=mybir.AxisListType.X, negate=True
                )
                wexp = work.tile([128, LP], F32, tag="wexp")
                sumw = small.tile([128, 1], F32, tag="sumw")
                nc.scalar.activation(
                    out=wexp, in_=logits, func=ex.Exp, bias=negmax, scale=1.0,
                    accum_out=sumw,
                )
                rcs = small.tile([128, 1], F32, tag="rcs")
                nc.vector.reciprocal(out=rcs, in_=sumw)

                # ---- W~^T via PE transposes ----
                wt_a_ps = tr_ps.tile([128, 128], F32, tag="tr")
                nc.tensor.transpose(wt_a_ps, wexp[:, 0:L0], ident)
                wt_a = work.tile([128, 128], F32, tag="wt_a")
                nc.vector.tensor_copy(out=wt_a, in_=wt_a_ps)
                wt_b_ps = tr_ps.tile([128, 128], F32, tag="tr")
                nc.tensor.transpose(wt_b_ps[0:L1, :], wexp[:, L0:LP], ident)
                wt_b = work.tile([L1, 128], F32, tag="wt_b")
                nc.vector.tensor_copy(out=wt_b, in_=wt_b_ps[0:L1, :])

                # ---- midT[i, (b k)] = sum_l lc[l,i] W~[k,l], per batch ----
                midT_ps = mb_ps.tile([DIN, 128], F32, tag="midT")
                for j in range(TB):
                    bj = h * TB + j
                    cols = slice(j * K, (j + 1) * K)
                    nc.tensor.matmul(
                        midT_ps[:, cols],
                        lhsT=tA[:, bj, :],
                        rhs=wt_a[:, cols],
                        start=(j == 0), stop=False,
                    )
                    nc.tensor.matmul(
                        midT_ps[:, cols],
                        lhsT=tB[:, bj, :],
                        rhs=wt_b[:, cols],
                        start=False, stop=(j == TB - 1),
                    )
                midT_sb = work.tile([DIN, 128], F32, tag="midT_sb")
                nc.vector.tensor_copy(out=midT_sb, in_=midT_ps)

                # ---- high_raw (unnormalized) = midT^T @ S ----
                high_ps = hi_ps.tile([128, DOUT], F32, tag="high")
                nc.tensor.matmul(
                    high_ps, lhsT=midT_sb, rhs=S_sb, start=True, stop=True
                )

                # ---- squash with softmax 1/sum folded in ----
                sq_scr = work.tile([128, DOUT], F32, tag="sq_scr")
                nsqU = small.tile([128, 1], F32, tag="nsqU")
                nc.scalar.activation(
                    out=sq_scr, in_=high_ps, func=ex.Square, accum_out=nsqU
                )
                rcs2 = small.tile([128, 1], F32, tag="rcs2")
                nc.vector.tensor_mul(rcs2, rcs, rcs)
                nsq = small.tile([128, 1], F32, tag="nsq")
                nc.vector.tensor_mul(nsq, nsqU, rcs2)
                t1 = small.tile([128, 1], F32, tag="t1")
                nc.scalar.activation(out=t1, in_=nsq, func=ex.Sqrt, bias=eps_t)
                t2 = small.tile([128, 1], F32, tag="t2")
                nc.vector.tensor_scalar_add(t2, nsq, 1.0)
                t3 = small.tile([128, 1], F32, tag="t3")
                nc.vector.tensor_mul(t3, t1, t2)
                rc2 = small.tile([128, 1], F32, tag="rc2")
                nc.vector.reciprocal(out=rc2, in_=t3)
                scl0 = small.tile([128, 1], F32, tag="scl0")
                nc.vector.tensor_mul(scl0, nsq, rc2)
                scl = small.tile([128, 1], F32, tag="scl")
                nc.vector.tensor_mul(scl, scl0, rcs)
                high_sb = work.tile([128, DOUT], F32, tag="high_sb")
                nc.vector.tensor_scalar_mul(high_sb, high_ps, scl)

                if last:
                    nc.sync.dma_start(
                        out=out[ht * TB : (ht + 1) * TB, :, :].rearrange(
                            "b k o -> (b k) o"
                        ),
                        in_=high_sb,
                    )
                else:
                    # ---- delta += (high @ S^T) @ lcT, pair-packed ----
                    highT_ps = tr_ps.tile([128, 128], F32, tag="tr")
                    nc.tensor.transpose(highT_ps[0:DOUT, :], high_sb, ident)
                    highT_sb = work.tile([DOUT, 128], F32, tag="highT_sb")
                    nc.vector.tensor_copy(out=highT_sb, in_=highT_ps[0:DOUT, :])
                    # gt_dup[0:64, c]   = G[c//8][i, c%8]   (even rows: batch c//8)
                    # gt_dup[64:128, c] = G[c//8 + 1][i, c%8] (shifted by one batch)
                    # so cols [16p, 16p+8) stack the pair (2p, 2p+1) on partitions.
                    gt_ps_t = ga_ps.tile([128, 128], F32, tag="gt_dup")
                    nc.tensor.matmul(
                        gt_ps_t[0:DIN, :], lhsT=ST_sb, rhs=highT_sb,
                        start=True, stop=True, skip_group_check=True,
                    )
                    nc.tensor.matmul(
                        gt_ps_t[DIN:128, 0:120], lhsT=ST_sb,
                        rhs=highT_sb[:, K:128],
                        start=True, stop=True, skip_group_check=True,
                        tile_position=(0, 64),
                    )
                    gt_dup = work.tile([128, 120], F32, tag="gt_dup_sb")
                    nc.vector.tensor_copy(out=gt_dup, in_=gt_ps_t[:, 0:120])
                    for p in range(TB // 2):
                        gp = (st * 2 + h) * (TB // 2) + p  # global pair index
                        even = gp % 2 == 0
                        dps = delta_psA if even else delta_psB
                        if even:
                            sflag, first_accA = first_accA, False
                        else:
                            sflag, first_accB = first_accB, False
                        nc.tensor.matmul(
                            dps,
                            lhsT=gt_dup[:, 16 * p : 16 * p + 8],
                            rhs=tT[:, h * (TB // 2) + p, :],
                            start=sflag,
                            stop=(gp >= npairs_tot - 2),
                        )

        if not last:
            # ---- AllReduce delta, update replicated Bc ----
            import concourse.bass as bass

            dA_sb = work.tile([K, LP], F32, tag="dA_sb")
            nc.vector.tensor_copy(out=dA_sb, in_=delta_psA)
            delta_sb = work.tile([K, LP], F32, tag="delta_sb")
            nc.vector.tensor_add(out=delta_sb, in0=dA_sb, in1=delta_psB)
            ib_small = dram.tile([K, LP], F32, tag="ib_small")
            nc.sync.dma_start(out=ib_small, in_=delta_sb)
            # replicate 16x in DRAM so the AR result is already partition-replicated
            ib_rep = dram.tile([128, LP], F32, tag="ib_rep")
            src_rep = bass.AP(
                tensor=ib_small.tensor,
                offset=ib_small.offset,
                ap=[[0, TB]] + [list(d) for d in ib_small.ap],
            )
            nc.sync.dma_start(
                out=ib_rep.rearrange("(j k) l -> j k l", k=K), in_=src_rep
            )
            ob_rep = dram.tile([128, LP], F32, tag="ob_rep")
            nc.gpsimd.collective_compute(
                "AllReduce",
                mybir.AluOpType.add,
                replica_groups=[list(range(ncores))],
                ins=[ib_rep.opt()],
                outs=[ob_rep.opt()],
            )
            delta_rep = bcrep_pool.tile([128, LP], F32, tag="delta_rep")
            nc.sync.dma_start(out=delta_rep, in_=ob_rep)
            if dumps is not None:
                nc.sync.dma_start(out=dumps[it], in_=delta_rep)
            bc_next = bcrep_pool.tile([128, LP], F32)
            nc.vector.tensor_add(out=bc_next, in0=bc_rep, in1=delta_rep)
            bc_rep = bc_next


_NC_CACHE = None


def _get_nc():
    global _NC_CACHE
    if _NC_CACHE is None:
        _NC_CACHE = _build_program()
    return _NC_CACHE


def _stage_inputs(low_capsule, seq_len, B_matrix, S_matrix):
    lc = np.asarray(low_capsule, dtype=np.float32)
    sl = np.asarray(seq_len, dtype=np.int32).reshape(B)
    Bm = np.asarray(B_matrix, dtype=np.float32).reshape(K, L)
    Sm = np.ascontiguousarray(np.asarray(S_matrix, dtype=np.float32))

    Bc0 = np.zeros((K, LP), dtype=np.float32)
    Bc0[:, :L] = Bm
    Bc0 = np.tile(Bc0, (TB, 1))  # [128, LP] partition-replicated
    ST = np.ascontiguousarray(Sm.T)
    iota = np.arange(LP)

    in_maps = []
    for c in range(NCORES):
        lcs = lc[c * BS : (c + 1) * BS]            # [BS, L, DIN]
        sls = sl[c * BS : (c + 1) * BS]
        lcP = np.zeros((BS, LP, DIN), dtype=np.float32)
        lcP[:, :L, :] = lcs
        lcLh = np.ascontiguousarray(lcP.transpose(1, 0, 2))   # [LP, BS, DIN]
        lcTh = np.ascontiguousarray(
            lcP.reshape(BS // 2, 2, LP, DIN).transpose(1, 3, 0, 2)
        ).reshape(2 * DIN, BS // 2, LP)
        m01 = (iota[None, :] < sls[:, None]).astype(np.uint8)  # 1=valid
        maskBK = np.repeat(m01, K, axis=0)                       # [BS*K, LP]
        in_maps.append(
            dict(lcL=lcLh, lcT=lcTh, maskBK=maskBK, Bc0=Bc0, S=Sm, ST=ST)
        )
    return in_maps


def kernel(low_capsule, seq_len, B_matrix, S_matrix, _trace=False, _tmpdir=None):
    nc = _get_nc()
    in_maps = _stage_inputs(low_capsule, seq_len, B_matrix, S_matrix)
    res = bass_utils.run_bass_kernel_spmd(
        nc, in_maps, core_ids=list(range(NCORES)), trace=_trace, tmpdir=_tmpdir
    )
    outp = np.concatenate(
        [res.results[c]["high"] for c in range(NCORES)], axis=0
    ).astype(np.float32)
    if _trace:
        kernel.last_results = res
    return outp
